# revision 8
# baseline (speedup 1.0000x reference)
"""DeepseekV3 MoE MLP (grouped ragged GEMM) on 8 Trainium2 NeuronCores.

Strategy: expert-parallel. 32 experts / 8 cores = 4 experts per core; each
core processes its experts' token groups (tokens arrive pre-sorted by
expert). Compute in bf16 (fp32 accumulation in PSUM), fp32 output.

Per-core pipeline, per expert (H=2048, I=1408, C tokens padded):
  stage 1:  gateT[i,t] = sum_h W1[h,i] * XT[h,t]   (W1 tile = lhsT, XT = rhs)
            upT  [i,t] = sum_h W2[h,i] * XT[h,t]
            h2T  [i,t] = silu(gateT) * upT          (ScalarE Silu + VectorE mul)
  stage 2:  down [t,h] = sum_i h2T[i,t] * W3[i,h]   (h2T tile = lhsT, W3 = rhs)

All operands are laid out host-side so every DMA is 128 partitions x
contiguous per-partition blocks; no on-device transposes anywhere.
"""

import numpy as np
import ml_dtypes

# Problem constants (hardcoded per contract).
E = 32          # experts
H = 2048        # hidden dim
I = 1408        # moe intermediate dim
N_CORES = 8
EPC = E // N_CORES  # experts per core
P = 128
HO = H // P     # 16 h-subtiles
IT = I // P     # 11 i-subtiles
HC = H // 512   # 4 output h-chunks of 512

BF16 = ml_dtypes.bfloat16

_PROGRAM_CACHE: dict = {}


def _build_program(C: int):
    """Build + compile the per-core Bass program for per-expert capacity C
    (multiple of 512). Returns (nc, meta)."""
    import concourse.bacc as bacc
    import concourse.mybir as mybir
    import concourse.tile as tile

    NT = C // 512   # stage-1 token chunks of 512
    TT = C // P     # stage-2 token tiles of 128

    nc = bacc.Bacc("TRN2", debug=False, num_devices=N_CORES)

    xt = nc.dram_tensor("xt", [EPC * NT, P, HO, 512], mybir.dt.bfloat16,
                        kind="ExternalInput").ap()
    w1 = nc.dram_tensor("w1", [EPC, IT, P, HO, P], mybir.dt.bfloat16,
                        kind="ExternalInput").ap()
    w2 = nc.dram_tensor("w2", [EPC, IT, P, HO, P], mybir.dt.bfloat16,
                        kind="ExternalInput").ap()
    w3 = nc.dram_tensor("w3", [EPC, HC, P, IT, 512], mybir.dt.bfloat16,
                        kind="ExternalInput").ap()
    out = nc.dram_tensor("out", [EPC * C, H], mybir.dt.float32,
                         kind="ExternalOutput").ap()

    with tile.TileContext(nc) as tc:
        with (
            tc.tile_pool(name="xt_pool", bufs=2) as xt_pool,
            tc.tile_pool(name="w12_pool", bufs=3) as w12_pool,
            tc.tile_pool(name="w3_pool", bufs=2) as w3_pool,
            tc.tile_pool(name="h2t_pool", bufs=2) as h2t_pool,
            tc.tile_pool(name="act_pool", bufs=3) as act_pool,
            tc.tile_pool(name="out_pool", bufs=4) as out_pool,
            tc.tile_pool(name="ps_g", bufs=2, space="PSUM") as ps_g,
            tc.tile_pool(name="ps_u", bufs=2, space="PSUM") as ps_u,
            tc.tile_pool(name="ps_d", bufs=3, space="PSUM") as ps_d,
        ):
            for e in range(EPC):
                # xt_tiles[tch][ho] -> ([P,512] AP, the bf16 token tile for
                # that h-subtile). Expert 0 is latency-critical (nothing else
                # in flight): interleave the first weight tiles with 4 quarter
                # XT chunks so the first matmuls start ~4us in. Later experts
                # prefetch during the previous expert's compute; one big DMA
                # keeps SP issue cost low.
                xt_tiles = []
                w_first = None
                if e == 0:
                    HQ = HO // 4  # 4 ho per quarter-chunk
                    w1_sb = w12_pool.tile([P, HO, P], mybir.dt.bfloat16, tag="w1")
                    qtiles = []
                    for tch in range(NT):
                        qtiles.append([
                            xt_pool.tile([P, HQ, 512], mybir.dt.bfloat16,
                                         tag=f"xt0_{tch}_{q}",
                                         name=f"xtq_{tch}_{q}")
                            for q in range(4)
                        ])
                    # issue order: w1, q0, w2, q1, q2, q3 ...
                    nc.sync.dma_start(out=w1_sb[:], in_=w1[e, 0])
                    nc.sync.dma_start(out=qtiles[0][0][:],
                                      in_=xt[e * NT, :, 0:HQ])
                    w2_sb = w12_pool.tile([P, HO, P], mybir.dt.bfloat16, tag="w2")
                    nc.sync.dma_start(out=w2_sb[:], in_=w2[e, 0])
                    w_first = (w1_sb, w2_sb)
                    for tch in range(NT):
                        for q in range(4):
                            if tch == 0 and q == 0:
                                continue
                            nc.sync.dma_start(
                                out=qtiles[tch][q][:],
                                in_=xt[e * NT + tch, :, q * HQ:(q + 1) * HQ])
                    for tch in range(NT):
                        xt_tiles.append([qtiles[tch][ho // HQ][:, ho % HQ]
                                         for ho in range(HO)])
                else:
                    for tch in range(NT):
                        t_sb = xt_pool.tile([P, HO, 512], mybir.dt.bfloat16,
                                            tag="xt")
                        nc.sync.dma_start(out=t_sb[:], in_=xt[e * NT + tch])
                        xt_tiles.append([t_sb[:, ho] for ho in range(HO)])

                h2t = h2t_pool.tile([P, IT, C], mybir.dt.bfloat16, tag="h2t")

                # ---- stage 1: gateT/upT + silu*mul -> h2T ----
                for it in range(IT):
                    if it == 0 and w_first is not None:
                        w1_sb, w2_sb = w_first
                    else:
                        w1_sb = w12_pool.tile([P, HO, P], mybir.dt.bfloat16,
                                              tag="w1")
                        nc.sync.dma_start(out=w1_sb[:], in_=w1[e, it])
                        w2_sb = w12_pool.tile([P, HO, P], mybir.dt.bfloat16,
                                              tag="w2")
                        nc.sync.dma_start(out=w2_sb[:], in_=w2[e, it])

                    for tch in range(NT):
                        pg = ps_g.tile([P, 512], mybir.dt.float32, tag="pg")
                        pu = ps_u.tile([P, 512], mybir.dt.float32, tag="pu")
                        for ho in range(HO):
                            nc.tensor.matmul(pg, w1_sb[:, ho], xt_tiles[tch][ho],
                                             start=(ho == 0), stop=(ho == HO - 1))
                        for ho in range(HO):
                            nc.tensor.matmul(pu, w2_sb[:, ho], xt_tiles[tch][ho],
                                             start=(ho == 0), stop=(ho == HO - 1))
                        sil = act_pool.tile([P, 512], mybir.dt.float32, tag="sil")
                        nc.scalar.activation(sil, pg,
                                             mybir.ActivationFunctionType.Silu)
                        nc.vector.tensor_mul(
                            h2t[:, it, tch * 512:(tch + 1) * 512], sil, pu)

                # ---- stage 2: down = h2 @ W3 ----
                for hc in range(HC):
                    w3_sb = w3_pool.tile([P, IT, 512], mybir.dt.bfloat16, tag="w3")
                    nc.sync.dma_start(out=w3_sb[:], in_=w3[e, hc])
                    for tt in range(TT):
                        pd = ps_d.tile([P, 512], mybir.dt.float32, tag="pd")
                        for io in range(IT):
                            nc.tensor.matmul(
                                pd, h2t[:, io, tt * P:(tt + 1) * P], w3_sb[:, io],
                                start=(io == 0), stop=(io == IT - 1))
                        ot = out_pool.tile([P, 512], mybir.dt.float32, tag="ot")
                        nc.scalar.copy(ot, pd)
                        nc.sync.dma_start(
                            out=out[e * C + tt * P: e * C + (tt + 1) * P,
                                    hc * 512:(hc + 1) * 512],
                            in_=ot[:])

    nc.compile()
    return nc


def _prep_inputs(hidden_states, gate_w, up_w, down_w, group_sizes, C):
    """Host-side: group tokens by expert (padded to C), transpose, convert to
    bf16, and pre-tile everything into the DMA layouts the program expects.
    Returns (in_maps, offsets)."""
    T = hidden_states.shape[0]
    gs = np.asarray(group_sizes, dtype=np.int64)
    offsets = np.zeros(E + 1, dtype=np.int64)
    np.cumsum(gs, out=offsets[1:])
    assert offsets[-1] == T, f"group_sizes sum {offsets[-1]} != T {T}"

    # Pad each expert's token block to C rows, convert to bf16.
    x_pad = np.zeros((E, C, H), dtype=BF16)
    for e in range(E):
        x_pad[e, :gs[e]] = hidden_states[offsets[e]:offsets[e + 1]]

    NT = C // 512
    # XT layout: [core][e_local*NT + tch][p][ho][512] with h = ho*128 + p
    # x_pad: [E, C, H] -> [E, NT, 512, HO, P] -> transpose to [E, NT, P, HO, 512]
    xt_all = np.ascontiguousarray(
        x_pad.reshape(E, NT, 512, HO, P).transpose(0, 1, 4, 3, 2)
    ).reshape(N_CORES, EPC * NT, P, HO, 512)

    # W1/W2 layout: [E][it][p][ho][128i] with h = ho*128 + p
    def tile_w12(w):
        wb = np.asarray(w, dtype=BF16)
        return np.ascontiguousarray(
            wb.reshape(E, HO, P, IT, P).transpose(0, 3, 2, 1, 4)
        ).reshape(N_CORES, EPC, IT, P, HO, P)

    w1_all = tile_w12(gate_w)
    w2_all = tile_w12(up_w)

    # W3 layout: [E][hc][p][io][512h] with i = io*128 + p
    w3b = np.asarray(down_w, dtype=BF16)
    w3_all = np.ascontiguousarray(
        w3b.reshape(E, IT, P, HC, 512).transpose(0, 3, 2, 1, 4)
    ).reshape(N_CORES, EPC, HC, P, IT, 512)

    in_maps = [
        {"xt": xt_all[c], "w1": w1_all[c], "w2": w2_all[c], "w3": w3_all[c]}
        for c in range(N_CORES)
    ]
    return in_maps, offsets, gs


LAST_RESULT = None  # stash for test harness trace analysis


def _run(hidden_states, gate_w, up_w, down_w, group_sizes, trace=False):
    from concourse.bass_utils import run_bass_kernel_spmd

    gs = np.asarray(group_sizes, dtype=np.int64)
    max_g = int(gs.max()) if gs.size else 512
    C = max(512, -(-max_g // 512) * 512)  # round up to multiple of 512

    key = ("v1", C)
    if key not in _PROGRAM_CACHE:
        _PROGRAM_CACHE[key] = _build_program(C)
    nc = _PROGRAM_CACHE[key]

    in_maps, offsets, gs = _prep_inputs(
        hidden_states, gate_w, up_w, down_w, group_sizes, C)

    res = run_bass_kernel_spmd(nc, in_maps, core_ids=list(range(N_CORES)),
                               trace=trace)
    global LAST_RESULT
    LAST_RESULT = res

    T = hidden_states.shape[0]
    out_full = np.empty((T, H), dtype=np.float32)
    for c in range(N_CORES):
        core_out = res.results[c]["out"]  # [EPC*C, H] fp32
        for el in range(EPC):
            e = c * EPC + el
            out_full[offsets[e]:offsets[e + 1]] = \
                core_out[el * C: el * C + gs[e]]
    return out_full, res.exec_time_ns


def kernel(hidden_states, gate_w, up_w, down_w, group_sizes):
    out, _ = _run(hidden_states, gate_w, up_w, down_w, group_sizes)
    return out



# revision 16
# speedup vs baseline: 1.0042x; 1.0042x over previous
"""DeepseekV3 MoE MLP (grouped ragged GEMM) on 8 Trainium2 NeuronCores.

Strategy: expert-parallel. 32 experts / 8 cores = 4 experts per core; each
core processes its experts' token groups (tokens arrive pre-sorted by
expert). Compute in bf16 (fp32 accumulation in PSUM), fp32 output.

Per-core pipeline, per expert (H=2048, I=1408, C tokens padded):
  stage 1:  gateT[i,t] = sum_h W1[h,i] * XT[h,t]   (W1 tile = lhsT, XT = rhs)
            upT  [i,t] = sum_h W2[h,i] * XT[h,t]
            h2T  [i,t] = silu(gateT) * upT          (ScalarE Silu + VectorE mul)
  stage 2:  down [t,h] = sum_i h2T[i,t] * W3[i,h]   (h2T tile = lhsT, W3 = rhs)

All operands are laid out host-side so every DMA is 128 partitions x
contiguous per-partition blocks; no on-device transposes anywhere.

TimelineSim: 460.7us/core, PE (tensor) engine ~98% busy with 0.6us of
idle after the first matmul -- i.e. at the bf16 matmul roofline (450.6us
of pure matmul cycles). The prologue is DMA-bound: expert 0's first
i-tile needs w1+w2+XT = 3MB landed (~8.3us at ~360GB/s), so its XT load
is split into 4 quarter chunks interleaved with the first weight tiles
(dma_start issue costs ~650ns on the SP sequencer, so fewer+bigger DMAs
win; per-ho splits or issuing XT from the Activation sequencer both
regressed). Dummy warmup matmuls ramp the PE p-state during the DMA
prologue; the final output unit is split in half to shorten the
epilogue's copy+DMA chain. fp8 was measured and rejected: e4m3 gives 6%
rel err vs the 2% gate (bf16: 0.41%).
"""

import numpy as np
import ml_dtypes

# Problem constants (hardcoded per contract).
E = 32          # experts
H = 2048        # hidden dim
I = 1408        # moe intermediate dim
N_CORES = 8
EPC = E // N_CORES  # experts per core
P = 128
HO = H // P     # 16 h-subtiles
IT = I // P     # 11 i-subtiles
HC = H // 512   # 4 output h-chunks of 512

BF16 = ml_dtypes.bfloat16

_PROGRAM_CACHE: dict = {}


def _build_program(C: int):
    """Build + compile the per-core Bass program for per-expert capacity C
    (multiple of 512). Returns (nc, meta)."""
    import concourse.bacc as bacc
    import concourse.mybir as mybir
    import concourse.tile as tile

    NT = C // 512   # stage-1 token chunks of 512
    TT = C // P     # stage-2 token tiles of 128

    nc = bacc.Bacc("TRN2", debug=False, num_devices=N_CORES)

    xt = nc.dram_tensor("xt", [EPC * NT, P, HO, 512], mybir.dt.bfloat16,
                        kind="ExternalInput").ap()
    w1 = nc.dram_tensor("w1", [EPC, IT, P, HO, P], mybir.dt.bfloat16,
                        kind="ExternalInput").ap()
    w2 = nc.dram_tensor("w2", [EPC, IT, P, HO, P], mybir.dt.bfloat16,
                        kind="ExternalInput").ap()
    w3 = nc.dram_tensor("w3", [EPC, HC, P, IT, 512], mybir.dt.bfloat16,
                        kind="ExternalInput").ap()
    out = nc.dram_tensor("out", [EPC * C, H], mybir.dt.float32,
                         kind="ExternalOutput").ap()

    with tile.TileContext(nc) as tc:
        with (
            tc.tile_pool(name="xt_pool", bufs=2) as xt_pool,
            tc.tile_pool(name="w12_pool", bufs=3) as w12_pool,
            tc.tile_pool(name="w3_pool", bufs=2) as w3_pool,
            tc.tile_pool(name="h2t_pool", bufs=2) as h2t_pool,
            tc.tile_pool(name="act_pool", bufs=3) as act_pool,
            tc.tile_pool(name="out_pool", bufs=4) as out_pool,
            tc.tile_pool(name="ps_g", bufs=2, space="PSUM") as ps_g,
            tc.tile_pool(name="ps_u", bufs=2, space="PSUM") as ps_u,
            tc.tile_pool(name="ps_d", bufs=3, space="PSUM") as ps_d,
            tc.tile_pool(name="ps_w", bufs=1, space="PSUM") as ps_w,
        ):
            # PE p-state warmup: the first ~5.8us are DMA-bound (PE idle), and
            # the tensor engine ramps 0.65->1.2->2.4GHz only after sustained
            # busy time. Chain small dummy matmuls on a zeroed tile during the
            # prologue so the real matmuls start at (or near) full clock.
            wz = act_pool.tile([P, P], mybir.dt.bfloat16, tag="warm", bufs=1)
            nc.vector.memset(wz[:], 0.0)
            pw = ps_w.tile([P, 128], mybir.dt.float32, tag="pw")
            for _ in range(36):
                nc.tensor.matmul(pw, wz[:], wz[:, :128], start=True, stop=True)

            for e in range(EPC):
                # xt_tiles[tch][ho] -> ([P,512] AP, the bf16 token tile for
                # that h-subtile). Expert 0 is latency-critical (nothing else
                # in flight): interleave the first weight tiles with 4 quarter
                # XT chunks so the first matmuls start ~4us in. Later experts
                # prefetch during the previous expert's compute; one big DMA
                # keeps SP issue cost low.
                xt_tiles = []
                w_first = None
                if e == 0:
                    HQ = HO // 4  # 4 ho per quarter-chunk
                    w1_sb = w12_pool.tile([P, HO, P], mybir.dt.bfloat16, tag="w1")
                    qtiles = []
                    for tch in range(NT):
                        qtiles.append([
                            xt_pool.tile([P, HQ, 512], mybir.dt.bfloat16,
                                         tag=f"xt0_{tch}_{q}",
                                         name=f"xtq_{tch}_{q}")
                            for q in range(4)
                        ])
                    # issue order: w1, q0, w2, q1, q2, q3 ...
                    nc.sync.dma_start(out=w1_sb[:], in_=w1[e, 0])
                    nc.sync.dma_start(out=qtiles[0][0][:],
                                      in_=xt[e * NT, :, 0:HQ])
                    w2_sb = w12_pool.tile([P, HO, P], mybir.dt.bfloat16, tag="w2")
                    nc.sync.dma_start(out=w2_sb[:], in_=w2[e, 0])
                    w_first = (w1_sb, w2_sb)
                    for tch in range(NT):
                        for q in range(4):
                            if tch == 0 and q == 0:
                                continue
                            nc.sync.dma_start(
                                out=qtiles[tch][q][:],
                                in_=xt[e * NT + tch, :, q * HQ:(q + 1) * HQ])
                    for tch in range(NT):
                        xt_tiles.append([qtiles[tch][ho // HQ][:, ho % HQ]
                                         for ho in range(HO)])
                else:
                    for tch in range(NT):
                        t_sb = xt_pool.tile([P, HO, 512], mybir.dt.bfloat16,
                                            tag="xt")
                        nc.sync.dma_start(out=t_sb[:], in_=xt[e * NT + tch])
                        xt_tiles.append([t_sb[:, ho] for ho in range(HO)])

                h2t = h2t_pool.tile([P, IT, C], mybir.dt.bfloat16, tag="h2t")

                # ---- stage 1: gateT/upT + silu*mul -> h2T ----
                for it in range(IT):
                    if it == 0 and w_first is not None:
                        w1_sb, w2_sb = w_first
                    else:
                        w1_sb = w12_pool.tile([P, HO, P], mybir.dt.bfloat16,
                                              tag="w1")
                        nc.sync.dma_start(out=w1_sb[:], in_=w1[e, it])
                        w2_sb = w12_pool.tile([P, HO, P], mybir.dt.bfloat16,
                                              tag="w2")
                        nc.sync.dma_start(out=w2_sb[:], in_=w2[e, it])

                    for tch in range(NT):
                        pg = ps_g.tile([P, 512], mybir.dt.float32, tag="pg")
                        pu = ps_u.tile([P, 512], mybir.dt.float32, tag="pu")
                        for ho in range(HO):
                            nc.tensor.matmul(pg, w1_sb[:, ho], xt_tiles[tch][ho],
                                             start=(ho == 0), stop=(ho == HO - 1))
                        for ho in range(HO):
                            nc.tensor.matmul(pu, w2_sb[:, ho], xt_tiles[tch][ho],
                                             start=(ho == 0), stop=(ho == HO - 1))
                        sil = act_pool.tile([P, 512], mybir.dt.float32, tag="sil")
                        nc.scalar.activation(sil, pg,
                                             mybir.ActivationFunctionType.Silu)
                        nc.vector.tensor_mul(
                            h2t[:, it, tch * 512:(tch + 1) * 512], sil, pu)

                # ---- stage 2: down = h2 @ W3 ----
                TT_ = C // P
                for hc in range(HC):
                    w3_sb = w3_pool.tile([P, IT, 512], mybir.dt.bfloat16, tag="w3")
                    nc.sync.dma_start(out=w3_sb[:], in_=w3[e, hc])
                    for tt in range(TT):
                        # The very last (expert, hc, tt) unit is the epilogue
                        # critical path: split its copy+DMA chain in half so
                        # the post-last-matmul latency is ~halved.
                        last = (e == EPC - 1 and hc == HC - 1 and tt == TT_ - 1)
                        splits = ((0, 256), (256, 512)) if last else ((0, 512),)
                        for lo, hi in splits:
                            # the halves of the final unit borrow the stage-1
                            # PSUM rings (idle by now) to stay within 8 banks
                            psp = ps_d if not last else (ps_g if lo == 0 else ps_u)
                            pd = psp.tile([P, hi - lo], mybir.dt.float32,
                                          tag="pd" if not last else
                                          ("pg" if lo == 0 else "pu"),
                                          name=f"pd_{lo}")
                            for io in range(IT):
                                nc.tensor.matmul(
                                    pd, h2t[:, io, tt * P:(tt + 1) * P],
                                    w3_sb[:, io, lo:hi],
                                    start=(io == 0), stop=(io == IT - 1))
                            ot = out_pool.tile([P, hi - lo], mybir.dt.float32,
                                               tag="ot" if not last else f"otl{lo}",
                                               bufs=None if not last else 1,
                                               name=f"ot_{lo}")
                            nc.scalar.copy(ot, pd)
                            nc.sync.dma_start(
                                out=out[e * C + tt * P: e * C + (tt + 1) * P,
                                        hc * 512 + lo:hc * 512 + hi],
                                in_=ot[:])

    nc.compile()
    return nc


def _prep_inputs(hidden_states, gate_w, up_w, down_w, group_sizes, C):
    """Host-side: group tokens by expert (padded to C), transpose, convert to
    bf16, and pre-tile everything into the DMA layouts the program expects.
    Returns (in_maps, offsets)."""
    T = hidden_states.shape[0]
    gs = np.asarray(group_sizes, dtype=np.int64)
    offsets = np.zeros(E + 1, dtype=np.int64)
    np.cumsum(gs, out=offsets[1:])
    assert offsets[-1] == T, f"group_sizes sum {offsets[-1]} != T {T}"

    # Pad each expert's token block to C rows, convert to bf16.
    x_pad = np.zeros((E, C, H), dtype=BF16)
    for e in range(E):
        x_pad[e, :gs[e]] = hidden_states[offsets[e]:offsets[e + 1]]

    NT = C // 512
    # XT layout: [core][e_local*NT + tch][p][ho][512] with h = ho*128 + p
    # x_pad: [E, C, H] -> [E, NT, 512, HO, P] -> transpose to [E, NT, P, HO, 512]
    xt_all = np.ascontiguousarray(
        x_pad.reshape(E, NT, 512, HO, P).transpose(0, 1, 4, 3, 2)
    ).reshape(N_CORES, EPC * NT, P, HO, 512)

    # W1/W2 layout: [E][it][p][ho][128i] with h = ho*128 + p
    def tile_w12(w):
        wb = np.asarray(w, dtype=BF16)
        return np.ascontiguousarray(
            wb.reshape(E, HO, P, IT, P).transpose(0, 3, 2, 1, 4)
        ).reshape(N_CORES, EPC, IT, P, HO, P)

    w1_all = tile_w12(gate_w)
    w2_all = tile_w12(up_w)

    # W3 layout: [E][hc][p][io][512h] with i = io*128 + p
    w3b = np.asarray(down_w, dtype=BF16)
    w3_all = np.ascontiguousarray(
        w3b.reshape(E, IT, P, HC, 512).transpose(0, 3, 2, 1, 4)
    ).reshape(N_CORES, EPC, HC, P, IT, 512)

    in_maps = [
        {"xt": xt_all[c], "w1": w1_all[c], "w2": w2_all[c], "w3": w3_all[c]}
        for c in range(N_CORES)
    ]
    return in_maps, offsets, gs


LAST_RESULT = None  # stash for test harness trace analysis


def _run(hidden_states, gate_w, up_w, down_w, group_sizes, trace=False):
    from concourse.bass_utils import run_bass_kernel_spmd

    gs = np.asarray(group_sizes, dtype=np.int64)
    max_g = int(gs.max()) if gs.size else 512
    C = max(512, -(-max_g // 512) * 512)  # round up to multiple of 512

    key = ("v1", C)
    if key not in _PROGRAM_CACHE:
        _PROGRAM_CACHE[key] = _build_program(C)
    nc = _PROGRAM_CACHE[key]

    in_maps, offsets, gs = _prep_inputs(
        hidden_states, gate_w, up_w, down_w, group_sizes, C)

    res = run_bass_kernel_spmd(nc, in_maps, core_ids=list(range(N_CORES)),
                               trace=trace)
    global LAST_RESULT
    LAST_RESULT = res

    T = hidden_states.shape[0]
    out_full = np.empty((T, H), dtype=np.float32)
    for c in range(N_CORES):
        core_out = res.results[c]["out"]  # [EPC*C, H] fp32
        for el in range(EPC):
            e = c * EPC + el
            out_full[offsets[e]:offsets[e + 1]] = \
                core_out[el * C: el * C + gs[e]]
    return out_full, res.exec_time_ns


def kernel(hidden_states, gate_w, up_w, down_w, group_sizes):
    out, _ = _run(hidden_states, gate_w, up_w, down_w, group_sizes)
    return out

